# revision 5
# baseline (speedup 1.0000x reference)
"""Detection-criterion loss kernel for Trainium2 (8 NeuronCores, SPMD).

loss = 2*class_bce + 4*xywh_sse + obj_bce   summed over 6M (batch*anchor) rows.

Math: for a binary target t and prob p,
    t*log(p) + (1-t)*log(1-p) = log|p + t - 1|
Class terms (weight 2): q_j = (cls != j) - p_j, contribution -sum ln(q_j^2).
Obj term (weight 1): |q| = |obj + p - 1|, contribution -sum ln|q|
(= -0.5 sum ln q^2). Both ride ONE Ln pass over [q_cls^2 | |q_obj|] with a
single accumulator S; coords use Square with its own accumulator A:
    total = 4*A - S.

Per-core layout (12 tiles of 128x470 rows + 1 tile of 128x235):
    X[0:3R]  = d  (coords diff, interleaved)  GPSIMD tensor_sub
    X[6R:7R] = obj + p                        GPSIMD tensor_add
    X[3R:6R] = (cls != j) - p_j, planar       DVE stt x3
    Y[0:3R]  = Square(X[3R:6R]) bf16          ACT
    Y[3R:4R] = Abs(X[6R:7R] - 1) bf16         ACT (free affine bias=-1)
    Ln(Y[0:4R])     accum-> acc[:, T+t]       ACT (accum_out)
    Square(X[0:3R]) accum-> acc[:, t]         ACT (accum_out)
No PE matmuls: reductions ride activation accum_out; the [128, 2T] partials
are summed on the host in float64.
"""

import numpy as np

P = 128                    # SBUF partitions
RB = 470                   # rows per partition, big tile
NB = 12                    # big tiles
RL = 235                   # last (small) tile rows
T = NB + 1                 # 13 tiles per core
CORE_ROWS = P * (NB * RB + RL)  # 752000
N_CORES = 8
TOTAL_ROWS = 2_000_000 * 3

_CACHE = {}


def _build_module(io_bufs: int = 4, work_bufs: int = 2):
    import concourse.bacc as bacc
    import concourse.bass as bass
    import concourse.tile as tile
    from concourse import mybir

    f32 = mybir.dt.float32
    bf16 = mybir.dt.bfloat16
    AF = mybir.ActivationFunctionType
    OP = mybir.AluOpType

    nc = bacc.Bacc(None, target_bir_lowering=False)

    o_d = nc.dram_tensor("o", [CORE_ROWS, 7], f32, kind="ExternalInput")
    g_d = nc.dram_tensor("g", [CORE_ROWS, 5], f32, kind="ExternalInput")
    res_d = nc.dram_tensor("res", [P, 2 * T], f32, kind="ExternalOutput")

    with tile.TileContext(nc) as tc:
        with (
            tc.tile_pool(name="io", bufs=io_bufs) as io,
            tc.tile_pool(name="work", bufs=work_bufs) as work,
            tc.tile_pool(name="consts", bufs=1) as consts,
        ):
            acc = consts.tile([P, 2 * T], f32)
            neg1 = consts.tile([P, 1], f32)
            nc.vector.memset(neg1[:], -1.0)

            lo = 0
            for t in range(T):
                R = RB if t < NB else RL
                sfx = "" if t < NB else "_l"
                R3, R4, R6, R7 = 3 * R, 4 * R, 6 * R, 7 * R

                so = io.tile([P, R, 7], f32, tag="so" + sfx)
                sg = io.tile([P, R, 5], f32, tag="sg" + sfx)
                nc.sync.dma_start(
                    out=so[:],
                    in_=o_d[lo : lo + P * R, :].rearrange("(p j) c -> p j c", p=P),
                )
                nc.sync.dma_start(
                    out=sg[:],
                    in_=g_d[lo : lo + P * R, :].rearrange("(p j) c -> p j c", p=P),
                )
                lo += P * R

                x = work.tile([P, 7 * R], f32, tag="x" + sfx)
                y = work.tile([P, R4], bf16, tag="y" + sfx)
                scr = work.tile([P, R4], bf16, tag="scr" + sfx)

                # class terms: s_j = (cls != j) - p_j  (DVE)
                for j in range(3):
                    nc.vector.scalar_tensor_tensor(
                        out=x[:, R3 + j * R : R4 + j * R],
                        in0=sg[:, :, 4],
                        scalar=float(j),
                        in1=so[:, :, 4 + j],
                        op0=OP.not_equal,
                        op1=OP.subtract,
                    )
                # coord diffs, interleaved (GPSIMD)
                nc.gpsimd.tensor_sub(
                    x[:, 0:R3].rearrange("p (r c) -> p r c", c=3),
                    so[:, :, 1:4],
                    sg[:, :, 1:4],
                )
                # obj raw sum: obj + p  (GPSIMD); |obj + p - 1| = |q_obj|
                nc.gpsimd.tensor_add(x[:, R6:R7], sg[:, :, 0], so[:, :, 0])

                # q_cls^2 -> bf16
                nc.scalar.activation(y[:, 0:R3], x[:, R3:R6], AF.Square)
                # |q_obj| -> bf16 via Abs(x - 1)
                nc.scalar.activation(
                    y[:, R3:R4], x[:, R6:R7], AF.Abs, bias=neg1[:], scale=1.0
                )
                # S += sum ln over [q_cls^2 | |q_obj|]
                nc.scalar.activation(
                    scr[:], y[:], AF.Ln, accum_out=acc[:, T + t : T + t + 1]
                )
                # A += sum d^2
                nc.scalar.activation(
                    scr[:, 0:R3], x[:, 0:R3], AF.Square,
                    accum_out=acc[:, t : t + 1],
                )

            nc.sync.dma_start(res_d[:, :], acc[:])

    nc.compile()
    return nc


def _get_module(io_bufs: int = 4, work_bufs: int = 2):
    key = ("nc", io_bufs, work_bufs)
    if key not in _CACHE:
        _CACHE[key] = _build_module(io_bufs, work_bufs)
    return _CACHE[key]


def kernel(output: np.ndarray, target: np.ndarray) -> np.ndarray:
    from concourse.bass_utils import run_bass_kernel_spmd

    o = np.ascontiguousarray(output, dtype=np.float32).reshape(TOTAL_ROWS, 7)
    g = np.ascontiguousarray(target, dtype=np.float32).reshape(TOTAL_ROWS, 5)

    in_maps = []
    for c in range(N_CORES):
        lo = c * CORE_ROWS
        hi = min(lo + CORE_ROWS, TOTAL_ROWS)
        oc, gc = o[lo:hi], g[lo:hi]
        if hi - lo < CORE_ROWS:
            padn = CORE_ROWS - (hi - lo)
            opad = np.zeros((padn, 7), np.float32)
            gpad = np.zeros((padn, 5), np.float32)
            gpad[:, 4] = -1.0  # class id outside [0,3) -> zero loss contribution
            oc = np.concatenate([oc, opad])
            gc = np.concatenate([gc, gpad])
        in_maps.append({"o": oc, "g": gc})

    nc = _get_module()
    r = run_bass_kernel_spmd(nc, in_maps, core_ids=list(range(N_CORES)))

    total = 0.0
    for c in range(N_CORES):
        res = np.asarray(r.results[c]["res"]).astype(np.float64)
        sums = res.sum(axis=0)  # [2T]
        total += 4.0 * sums[0:T].sum() - sums[T : 2 * T].sum()
    return np.array(total, dtype=np.float32)


# revision 6
# speedup vs baseline: 1.0502x; 1.0502x over previous
"""Detection-criterion loss kernel for Trainium2 (8 NeuronCores, SPMD).

loss = 2*class_bce + 4*xywh_sse + obj_bce   summed over 6M (batch*anchor) rows.

Math: for a binary target t and prob p,
    t*log(p) + (1-t)*log(1-p) = log|p + t - 1|
Class terms (weight 2): q_j = (cls != j) - p_j, contribution -sum ln(q_j^2).
Obj term (weight 1): q = (obj == 0) - p, contribution -sum ln|q|
(= -0.5 sum ln q^2). Both ride ONE Ln pass over [q_cls^2 | |q_obj|] with a
single accumulator S; coords use Square with its own accumulator A:
    total = 4*A - S.

All elementwise prep runs on DVE (GPSIMD shares the DVE SBUF port at ~2x
worse per-element cost, so using it slows DVE more than it helps); all
squares/abs/ln + reductions run on ACT via accum_out. No PE matmuls. The
[128, 2T] per-tile partials are summed on the host in float64.

Per-core layout (12 tiles of 128x470 rows + 1 tile of 128x235):
    X[0:3R]  = d   (coords diff, interleaved)   DVE tensor_sub (bf16 out)
    X[3R:6R] = q_cls planar                     DVE stt x3     (bf16 out)
    X[6R:7R] = q_obj                            DVE stt        (bf16 out)
    Y[0:3R]  = Square(X[3R:6R]) bf16            ACT
    Y[3R:4R] = Abs(X[6R:7R]) bf16               ACT
    Ln(Y[0:4R])     accum-> acc[:, T+t]         ACT (accum_out)
    Square(X[0:3R]) accum-> acc[:, t]           ACT (accum_out)
"""

import numpy as np

P = 128                    # SBUF partitions
RB = 470                   # rows per partition, big tile
NB = 12                    # big tiles
RL = 235                   # last (small) tile rows
T = NB + 1                 # 13 tiles per core
CORE_ROWS = P * (NB * RB + RL)  # 752000
N_CORES = 8
TOTAL_ROWS = 2_000_000 * 3

_CACHE = {}


def _build_module(io_bufs: int = 5, work_bufs: int = 2):
    import concourse.bacc as bacc
    import concourse.bass as bass
    import concourse.tile as tile
    from concourse import mybir

    f32 = mybir.dt.float32
    bf16 = mybir.dt.bfloat16
    AF = mybir.ActivationFunctionType
    OP = mybir.AluOpType

    nc = bacc.Bacc(None, target_bir_lowering=False)

    o_d = nc.dram_tensor("o", [CORE_ROWS, 7], f32, kind="ExternalInput")
    g_d = nc.dram_tensor("g", [CORE_ROWS, 5], f32, kind="ExternalInput")
    res_d = nc.dram_tensor("res", [P, 2 * T], f32, kind="ExternalOutput")

    with tile.TileContext(nc) as tc:
        with (
            tc.tile_pool(name="io", bufs=io_bufs) as io,
            tc.tile_pool(name="work", bufs=work_bufs) as work,
            tc.tile_pool(name="consts", bufs=1) as consts,
        ):
            acc = consts.tile([P, 2 * T], f32)

            lo = 0
            for t in range(T):
                R = RB if t < NB else RL
                sfx = "" if t < NB else "_l"
                R3, R4, R6, R7 = 3 * R, 4 * R, 6 * R, 7 * R

                so = io.tile([P, R, 7], f32, tag="so" + sfx)
                sg = io.tile([P, R, 5], f32, tag="sg" + sfx)
                nc.sync.dma_start(
                    out=so[:],
                    in_=o_d[lo : lo + P * R, :].rearrange("(p j) c -> p j c", p=P),
                )
                nc.sync.dma_start(
                    out=sg[:],
                    in_=g_d[lo : lo + P * R, :].rearrange("(p j) c -> p j c", p=P),
                )
                lo += P * R

                x = work.tile([P, 7 * R], bf16, tag="x" + sfx)
                y = work.tile([P, R4], bf16, tag="y" + sfx)
                scr = work.tile([P, R4], bf16, tag="scr" + sfx)

                # class terms: q_j = (cls != j) - p_j  (DVE stt)
                for j in range(3):
                    nc.vector.scalar_tensor_tensor(
                        out=x[:, R3 + j * R : R4 + j * R],
                        in0=sg[:, :, 4],
                        scalar=float(j),
                        in1=so[:, :, 4 + j],
                        op0=OP.not_equal,
                        op1=OP.subtract,
                    )
                # obj: q = (obj == 0) - p  (DVE stt)
                nc.vector.scalar_tensor_tensor(
                    out=x[:, R6:R7],
                    in0=sg[:, :, 0],
                    scalar=0.0,
                    in1=so[:, :, 0],
                    op0=OP.is_equal,
                    op1=OP.subtract,
                )
                # coord diffs, interleaved, innermost-contiguous (DVE TT)
                nc.vector.tensor_sub(
                    x[:, 0:R3].rearrange("p (r c) -> p r c", c=3),
                    so[:, :, 1:4],
                    sg[:, :, 1:4],
                )

                # q_cls^2 -> bf16
                nc.scalar.activation(y[:, 0:R3], x[:, R3:R6], AF.Square)
                # |q_obj| -> bf16
                nc.scalar.activation(y[:, R3:R4], x[:, R6:R7], AF.Abs)
                # S += sum ln over [q_cls^2 | |q_obj|]
                nc.scalar.activation(
                    scr[:], y[:], AF.Ln, accum_out=acc[:, T + t : T + t + 1]
                )
                # A += sum d^2
                nc.scalar.activation(
                    scr[:, 0:R3], x[:, 0:R3], AF.Square,
                    accum_out=acc[:, t : t + 1],
                )

            nc.sync.dma_start(res_d[:, :], acc[:])

    nc.compile()
    return nc


def _get_module(io_bufs: int = 5, work_bufs: int = 2):
    key = ("nc", io_bufs, work_bufs)
    if key not in _CACHE:
        _CACHE[key] = _build_module(io_bufs, work_bufs)
    return _CACHE[key]


def kernel(output: np.ndarray, target: np.ndarray) -> np.ndarray:
    from concourse.bass_utils import run_bass_kernel_spmd

    o = np.ascontiguousarray(output, dtype=np.float32).reshape(TOTAL_ROWS, 7)
    g = np.ascontiguousarray(target, dtype=np.float32).reshape(TOTAL_ROWS, 5)

    in_maps = []
    for c in range(N_CORES):
        lo = c * CORE_ROWS
        hi = min(lo + CORE_ROWS, TOTAL_ROWS)
        oc, gc = o[lo:hi], g[lo:hi]
        if hi - lo < CORE_ROWS:
            padn = CORE_ROWS - (hi - lo)
            opad = np.zeros((padn, 7), np.float32)
            gpad = np.zeros((padn, 5), np.float32)
            gpad[:, 4] = -1.0  # class id outside [0,3) -> zero loss contribution
            oc = np.concatenate([oc, opad])
            gc = np.concatenate([gc, gpad])
        in_maps.append({"o": oc, "g": gc})

    nc = _get_module()
    r = run_bass_kernel_spmd(nc, in_maps, core_ids=list(range(N_CORES)))

    total = 0.0
    for c in range(N_CORES):
        res = np.asarray(r.results[c]["res"]).astype(np.float64)
        sums = res.sum(axis=0)  # [2T]
        total += 4.0 * sums[0:T].sum() - sums[T : 2 * T].sum()
    return np.array(total, dtype=np.float32)


# revision 9
# speedup vs baseline: 1.1567x; 1.1014x over previous
"""Detection-criterion loss kernel for Trainium2 (8 NeuronCores, SPMD).

loss = 2*class_bce + 4*xywh_sse + obj_bce   summed over 6M (batch*anchor) rows.

Math: for a binary target t and prob p,
    t*log(p) + (1-t)*log(1-p) = log|p + t - 1|
Class terms (weight 2): q_j = (cls != j) - p_j, contribution -sum ln(q_j^2).
Obj term (weight 1): q = (obj == 0) - p, contribution -sum ln|q|
(= -0.5 sum ln q^2). Both ride ONE Ln pass over [q_cls^2 | |q_obj|] with a
single accumulator S; coords use Square with its own accumulator A:
    total = 4*A - S.

All elementwise prep runs on DVE (GPSIMD shares the DVE SBUF port at ~2x
worse per-element cost, so using it slows DVE more than it helps); all
squares/abs/ln + reductions run on ACT via accum_out. No PE matmuls. The
[128, 2T] per-tile partials are summed on the host in float64.

Per-core layout (12 tiles of 128x470 rows + 1 tile of 128x235):
    X[0:3R]  = d   (coords diff, interleaved)   DVE tensor_sub (bf16 out)
    X[3R:6R] = q_cls planar                     DVE stt x3     (bf16 out)
    X[6R:7R] = q_obj                            DVE stt        (bf16 out)
    Y[0:3R]  = Square(X[3R:6R]) bf16            ACT
    Y[3R:4R] = Abs(X[6R:7R]) bf16               ACT
    Ln(Y[0:4R])     accum-> acc[:, T+t]         ACT (accum_out)
    Square(X[0:3R]) accum-> acc[:, t]           ACT (accum_out)
"""

import numpy as np

P = 128                    # SBUF partitions
RB = 470                   # rows per partition, big tile
NB = 12                    # big tiles
RL = 235                   # last (small) tile rows
T = NB + 1                 # 13 tiles per core
CORE_ROWS = P * (NB * RB + RL)  # 752000
N_CORES = 8
TOTAL_ROWS = 2_000_000 * 3

_CACHE = {}


def _build_module(io_bufs: int = 4, work_bufs: int = 3):
    import concourse.bacc as bacc
    import concourse.bass as bass
    import concourse.tile as tile
    from concourse import mybir

    f32 = mybir.dt.float32
    bf16 = mybir.dt.bfloat16
    AF = mybir.ActivationFunctionType
    OP = mybir.AluOpType

    nc = bacc.Bacc(None, target_bir_lowering=False)

    o_d = nc.dram_tensor("o", [CORE_ROWS, 7], f32, kind="ExternalInput")
    g_d = nc.dram_tensor("g", [CORE_ROWS, 5], f32, kind="ExternalInput")
    res_d = nc.dram_tensor("res", [P, 2 * T], f32, kind="ExternalOutput")

    with tile.TileContext(nc) as tc:
        with (
            tc.tile_pool(name="io", bufs=io_bufs) as io,
            tc.tile_pool(name="work", bufs=work_bufs) as work,
            tc.tile_pool(name="consts", bufs=1) as consts,
        ):
            acc = consts.tile([P, 2 * T], f32)

            lo = 0
            for t in range(T):
                R = RB if t < NB else RL
                sfx = "" if t < NB else "_l"
                R3, R4, R6, R7 = 3 * R, 4 * R, 6 * R, 7 * R

                so = io.tile([P, R, 7], f32, tag="so" + sfx)
                sg = io.tile([P, R, 5], f32, tag="sg" + sfx)
                nc.sync.dma_start(
                    out=so[:],
                    in_=o_d[lo : lo + P * R, :].rearrange("(p j) c -> p j c", p=P),
                )
                nc.sync.dma_start(
                    out=sg[:],
                    in_=g_d[lo : lo + P * R, :].rearrange("(p j) c -> p j c", p=P),
                )
                lo += P * R

                xq = work.tile([P, R4], bf16, tag="xq" + sfx)
                xd = work.tile([P, R3], bf16, tag="xd" + sfx)
                y = work.tile([P, R4], bf16, tag="y" + sfx)
                scr = work.tile([P, R4], bf16, tag="scr" + sfx)

                # class terms: q_j = (cls != j) - p_j  (DVE stt)
                for j in range(3):
                    nc.vector.scalar_tensor_tensor(
                        out=xq[:, j * R : (j + 1) * R],
                        in0=sg[:, :, 4],
                        scalar=float(j),
                        in1=so[:, :, 4 + j],
                        op0=OP.not_equal,
                        op1=OP.subtract,
                    )
                # obj: q = (obj == 0) - p  (DVE stt)
                nc.vector.scalar_tensor_tensor(
                    out=xq[:, R3:R4],
                    in0=sg[:, :, 0],
                    scalar=0.0,
                    in1=so[:, :, 0],
                    op0=OP.is_equal,
                    op1=OP.subtract,
                )
                # coord diffs, interleaved, innermost-contiguous (DVE TT)
                nc.vector.tensor_sub(
                    xd[:].rearrange("p (r c) -> p r c", c=3),
                    so[:, :, 1:4],
                    sg[:, :, 1:4],
                )

                # q_cls^2 -> bf16
                nc.scalar.activation(y[:, 0:R3], xq[:, 0:R3], AF.Square)
                # |q_obj| -> bf16
                nc.scalar.activation(y[:, R3:R4], xq[:, R3:R4], AF.Abs)
                # S += sum ln over [q_cls^2 | |q_obj|]
                nc.scalar.activation(
                    scr[:], y[:], AF.Ln, accum_out=acc[:, T + t : T + t + 1]
                )
                # A += sum d^2
                nc.scalar.activation(
                    scr[:, 0:R3], xd[:], AF.Square,
                    accum_out=acc[:, t : t + 1],
                )

            nc.sync.dma_start(res_d[:, :], acc[:])

    nc.compile()
    return nc


def _get_module(io_bufs: int = 4, work_bufs: int = 3):
    key = ("nc", io_bufs, work_bufs)
    if key not in _CACHE:
        _CACHE[key] = _build_module(io_bufs, work_bufs)
    return _CACHE[key]


def kernel(output: np.ndarray, target: np.ndarray) -> np.ndarray:
    from concourse.bass_utils import run_bass_kernel_spmd

    o = np.ascontiguousarray(output, dtype=np.float32).reshape(TOTAL_ROWS, 7)
    g = np.ascontiguousarray(target, dtype=np.float32).reshape(TOTAL_ROWS, 5)

    in_maps = []
    for c in range(N_CORES):
        lo = c * CORE_ROWS
        hi = min(lo + CORE_ROWS, TOTAL_ROWS)
        oc, gc = o[lo:hi], g[lo:hi]
        if hi - lo < CORE_ROWS:
            padn = CORE_ROWS - (hi - lo)
            opad = np.zeros((padn, 7), np.float32)
            gpad = np.zeros((padn, 5), np.float32)
            gpad[:, 4] = -1.0  # class id outside [0,3) -> zero loss contribution
            oc = np.concatenate([oc, opad])
            gc = np.concatenate([gc, gpad])
        in_maps.append({"o": oc, "g": gc})

    nc = _get_module()
    r = run_bass_kernel_spmd(nc, in_maps, core_ids=list(range(N_CORES)))

    total = 0.0
    for c in range(N_CORES):
        res = np.asarray(r.results[c]["res"]).astype(np.float64)
        sums = res.sum(axis=0)  # [2T]
        total += 4.0 * sums[0:T].sum() - sums[T : 2 * T].sum()
    return np.array(total, dtype=np.float32)
